# revision 25
# baseline (speedup 1.0000x reference)
"""Binarized AlexNet on 8 Trainium2 NeuronCores (SPMD, data-parallel over batch).

- Batch 128 sharded 16 imgs/core; weights replicated.
- conv1: 3-pass fp16 (hi/lo split) matmuls over host im2col (error ~1e-6 so the
  bn1 sign decisions match the fp32 reference; bn1 boundary found by bit-exact
  fp32 emulation).
- Binarized activations kept as u in {0,1} fp8: pad(-1) maps to u=0, so conv
  inputs live on zero-padded flat grids; DoubleRow slots are row-shifts /
  channel-group interleaves; sums are exact integers in fp32 PSUM.
- Binarize decisions via host-precomputed per-channel integer thresholds that
  exactly emulate the reference's fp32 bn rounding (validated: 0 mismatches
  over all (channel, attainable-integer) pairs). Half-offset -> never ties.
- fc1/fc2: activation-stationary fp8 DR matmuls; thresholds via rank-1 (K=2
  fp16) matmuls; PE transposes between layers; bn7 folded into ACT Relu; fc3
  in fp16.
- DMA strategy: resident weights packed host-side into 5 group blobs (one DMA
  each); im2col tiles are single contiguous DMAs; fc1/fc2 weights stream as
  8 chunk DMAs each (contiguous 4.7MB / 2.1MB) double-buffered, fc3 as one
  8MB DMA, spread across the gpsimd/sync/scalar DMA queues so weight
  streaming overlaps compute instead of bottlenecking on descriptor issue.
"""
import dataclasses
import numpy as np
import ml_dtypes

import bass_rust
import concourse.bass as bass
import concourse.tile as tile
from concourse import mybir
from concourse.bass_utils import run_bass_kernel_spmd
from concourse.tile_rust import add_dep_helper

FP8 = ml_dtypes.float8_e4m3
BF16 = ml_dtypes.bfloat16

EPS = np.float32(1e-5)
NCORES = 8
DR = mybir.MatmulPerfMode.DoubleRow

# grids: (stride, base, tile-size, out-flat-count)
# S2/S4 carry extra zero rows so the all-zero phantom DoubleRow slot of the
# last conv2 / conv4/5 tap row stays in-bounds (reads zeros, weight is 0).
G2, B2, S2, N2 = 32, 66, 1152, 27 * 32          # conv2 in-grid (27x27 interior)
G3, B3, S3, N3 = 15, 16, 240, 13 * 15           # conv3 in-grid (13x13)
G4, B4, S4, N4 = 16, 17, 272, 13 * 16           # conv4/5 in-grid (13x13)


def apv(base, offset_add, dims):
    return dataclasses.replace(base, offset=base.offset + offset_add, ap=dims)


# ---- packed resident-weight group layouts (host layout == SBUF layout) ----

def _layout(entries):
    off, o = {}, 0
    for k, n in entries:
        off[k] = o
        o += n
    return off, o


GA_ENTRIES = (
    [(f'w3dr{mt}{ky}{kx}', 256) for mt in range(3) for ky in range(3) for kx in range(3)]
    + [(f'w4a{mt}{ky}{kx}', 256) for mt in range(3) for ky in range(3) for kx in range(3)]
    + [(f'w4bd{mt}{kx}', 256) for mt in range(3) for kx in range(3)]
    + [(f'w4bs{mt}{kx}', 256) for mt in range(3) for kx in range(3)]
    + [(f'w5a{mt}{ky}{kx}', 256) for mt in range(2) for ky in range(3) for kx in range(3)]
    + [(f'w5bd{mt}{kx}', 256) for mt in range(2) for kx in range(3)]
    + [(f'w5bs{mt}{kx}', 256) for mt in range(2) for kx in range(3)])
GA_OFF, GA_LEN = _layout(GA_ENTRIES)            # fp8, 128 rows
GB_ENTRIES = (
    [(f'w2dr{mt}{kx}{kg}', 256) for mt in range(2) for kx in range(5) for kg in range(2)]
    + [(f'w2sr{mt}{kx}', 256) for mt in range(2) for kx in range(5)])
GB_OFF, GB_LEN = _layout(GB_ENTRIES)            # fp8, 96 rows
GC_ENTRIES = ([(f'w1_hi{ct}', 96) for ct in range(3)]
              + [(f'w1_lo{ct}', 96) for ct in range(3)])
GC_OFF, GC_LEN = _layout(GC_ENTRIES)            # fp16, 121 rows
GD_ENTRIES = [('t1x', 1), ('thr2u', 2), ('thr3u', 3), ('thr4u', 3), ('thr5u', 2),
              ('sc7', 32), ('bi7', 32), ('id16f', 16)]
GD_OFF, GD_LEN = _layout(GD_ENTRIES)            # f32, 128 rows
GE_ENTRIES = [('thr6', 4096), ('b8', 1000)]
GE_OFF, GE_LEN = _layout(GE_ENTRIES)            # f16, 2 rows


# ======================= host-side preparation =======================

def _f32(x):
    return np.asarray(x, dtype=np.float32)


def bn_scale(bnp):
    g, b, m, v = [_f32(a) for a in bnp]
    s = (g / np.sqrt(v + EPS)).astype(np.float32)
    return s, b, m


def exact_bn1_threshold(bnp):
    """Per-channel minimal fp32 x with ((x-m)*s)+b >= 0 under fp32 rounding."""
    s, b, m = bn_scale(bnp)
    assert np.all(s > 0)
    C = len(s)

    def dec(x):
        with np.errstate(over='ignore', invalid='ignore'):
            return ((x - m).astype(np.float32) * s).astype(np.float32) + b >= 0

    def key(f):  # monotone uint32 encoding of fp32 order
        i = f.view(np.uint32).astype(np.uint32)
        neg = (i >> 31).astype(bool)
        return np.where(neg, ~i, i | np.uint32(0x80000000)).astype(np.uint32)

    def unkey(k):
        k = k.astype(np.uint32)
        hi = (k >> 31).astype(bool)
        return np.where(hi, k & np.uint32(0x7FFFFFFF), ~k).astype(np.uint32).view(np.float32)

    lo = np.full(C, -3.0e38, np.float32)
    hi = np.full(C, 3.0e38, np.float32)
    always = dec(lo)
    never = ~dec(hi)
    klo, khi = key(lo).astype(np.uint64), key(hi).astype(np.uint64)
    for _ in range(40):
        kmid = ((klo + khi) // 2).astype(np.uint64)
        d = dec(unkey(kmid.astype(np.uint32)))
        khi = np.where(d, kmid, khi)
        klo = np.where(d, klo, kmid)
    t = unkey(khi.astype(np.uint32))
    t = np.where(always, np.float32(-3.0e38), t)
    t = np.where(never, np.float32(3.0e38), t)
    chk = dec(t) | never
    assert np.all(chk), "bn1 threshold search failed"
    return t.astype(np.float32)


def integer_threshold(bnp, rmax):
    """Min even integer E in [-rmax,rmax] with fp32-bn(E) >= 0, per channel."""
    s, b, m = bn_scale(bnp)
    assert np.all(s > 0)
    C = len(s)
    T = np.zeros(C, np.float64)
    E = np.arange(-rmax, rmax + 1, 2, dtype=np.float32)
    for c0 in range(0, C, 512):
        c1 = min(C, c0 + 512)
        pre = ((E[None, :] - m[c0:c1, None]).astype(np.float32)
               * s[c0:c1, None]).astype(np.float32) + b[c0:c1, None]
        d = pre >= 0
        anyd = d.any(axis=1)
        t = E[np.argmax(d, axis=1)].astype(np.float64)
        t = np.where(anyd, t, rmax + 2)
        t = np.where(d[:, 0], -rmax - 2, t)
        T[c0:c1] = t
    return T


def binarize_w(w):
    return np.where(_f32(w) >= 0, 1.0, -1.0).astype(np.float32)


def prep_host(inputs):
    P = {}
    x = _f32(inputs['x'])
    Bfull = x.shape[0]

    # ---- conv1 im2col (fp16 hi/lo), [B, 121, 3*3025] so one DMA per tile ----
    xp = np.zeros((Bfull, 3, 228, 228), np.float32)
    xp[:, :, 2:226, 2:226] = x
    xh = xp.astype(np.float16)
    xl = (xp - xh.astype(np.float32)).astype(np.float16)

    def im2col(a):
        s = a.strides
        v = np.lib.stride_tricks.as_strided(
            a, (Bfull, 3, 11, 11, 55, 55),
            (s[0], s[1], s[2], s[3], s[2] * 4, s[3] * 4))
        v = v.reshape(Bfull, 3, 121, 3025).transpose(0, 2, 1, 3)
        return np.ascontiguousarray(v).reshape(Bfull, 121, 3 * 3025)

    P['col_hi'] = im2col(xh)
    P['col_lo'] = im2col(xl)

    w1 = _f32(inputs['conv1_w']).reshape(96, 3, 121)
    w1h = w1.astype(np.float16)
    w1l = (w1 - w1h.astype(np.float32)).astype(np.float16)
    w1ht = w1h.transpose(1, 2, 0)                # [3,121,96]
    w1lt = w1l.transpose(1, 2, 0)

    gC = np.zeros((121, GC_LEN), np.float16)
    for ct in range(3):
        gC[:, GC_OFF[f'w1_hi{ct}']:GC_OFF[f'w1_hi{ct}'] + 96] = w1ht[ct]
        gC[:, GC_OFF[f'w1_lo{ct}']:GC_OFF[f'w1_lo{ct}'] + 96] = w1lt[ct]
    P['gC'] = gC

    t1x = exact_bn1_threshold(inputs['bn1'])     # [96]

    w2 = binarize_w(inputs['bconv2_w'])
    w3 = binarize_w(inputs['bconv3_w'])
    w4 = binarize_w(inputs['bconv4_w'])
    w5 = binarize_w(inputs['bconv5_w'])
    w6 = binarize_w(inputs['blin1_w'])
    w7 = binarize_w(inputs['blin2_w'])

    gB = np.zeros((96, GB_LEN), np.float32)
    for mt in range(2):
        for kx in range(5):
            for kg in range(2):
                blk = np.zeros((96, 2, 128), np.float32)
                for j in range(2):
                    blk[:, j, :] = w2[mt * 128:(mt + 1) * 128, :, 2 * kg + j, kx].T
                o = GB_OFF[f'w2dr{mt}{kx}{kg}']
                gB[:, o:o + 256] = blk.reshape(96, 256)
            o = GB_OFF[f'w2sr{mt}{kx}']
            gB[:, o:o + 128] = w2[mt * 128:(mt + 1) * 128, :, 4, kx].T
    P['gB'] = gB.astype(FP8)

    K2 = w2.reshape(256, -1).sum(1)
    T2 = integer_threshold(inputs['bn2'], 2400)
    thr2u = ((T2 + K2) / 2 - 0.5).astype(np.float32).reshape(2, 128).T  # [128,2]

    gA = np.zeros((128, GA_LEN), np.float32)

    def put_dr(key, w, mt, ky, kx):
        blk = np.zeros((128, 2, 128), np.float32)
        for j in range(2):
            blk[:, j, :] = w[mt * 128:(mt + 1) * 128, 128 * j:128 * (j + 1), ky, kx].T
        o = GA_OFF[key]
        gA[:, o:o + 256] = blk.reshape(128, 256)

    for mt in range(3):
        for ky in range(3):
            for kx in range(3):
                put_dr(f'w3dr{mt}{ky}{kx}', w3, mt, ky, kx)

    K3 = w3.reshape(384, -1).sum(1)
    T3 = integer_threshold(inputs['bn3'], 2304)
    thr3u = ((T3 + K3) / 2 - 0.5).astype(np.float32).reshape(3, 128).T  # [128,3]

    def pack45(nm, w, nmt):
        for mt in range(nmt):
            for kx in range(3):
                for ky in range(3):
                    put_dr(f'{nm}a{mt}{ky}{kx}', w, mt, ky, kx)
                blk = np.zeros((128, 2, 128), np.float32)
                for j in range(2):
                    blk[:, j, :] = w[mt * 128:(mt + 1) * 128, 256:384, j, kx].T
                o = GA_OFF[f'{nm}bd{mt}{kx}']
                gA[:, o:o + 256] = blk.reshape(128, 256)
                o = GA_OFF[f'{nm}bs{mt}{kx}']
                gA[:, o:o + 128] = w[mt * 128:(mt + 1) * 128, 256:384, 2, kx].T

    pack45('w4', w4, 3)
    K4 = w4.reshape(384, -1).sum(1)
    T4 = integer_threshold(inputs['bn4'], 3456)
    thr4u = ((T4 + K4) / 2 - 0.5).astype(np.float32).reshape(3, 128).T

    pack45('w5', w5, 2)
    K5 = w5.reshape(256, -1).sum(1)
    T5 = integer_threshold(inputs['bn5'], 3456)
    thr5u = ((T5 + K5) / 2 - 0.5).astype(np.float32).reshape(2, 128).T

    P['gA'] = gA.astype(FP8)

    # fc1 rhs: w6r[s][p, j, n] = w6[n, (p+128j)*36 + s], chunked contiguous:
    # w6c[c, p, s*1024 + j*512 + nn] = w6r[s, p, j, 512c + nn]
    w6r = np.zeros((36, 128, 2, 4096), np.float32)
    for j in range(2):
        for s in range(36):
            cols = (np.arange(128) + 128 * j) * 36 + s
            w6r[s, :, j, :] = w6[:, cols].T
    P['w6c'] = np.ascontiguousarray(
        w6r.reshape(36, 128, 2, 8, 512).transpose(3, 1, 0, 2, 4)
    ).reshape(8, 128, 36 * 1024).astype(FP8)
    K6 = w6.sum(1)
    T6 = integer_threshold(inputs['bn6'], 9216)
    t6u = (T6 + K6) / 2 - 0.5
    t6hi = np.floor(t6u / 16) * 16
    t6lo = t6u - t6hi
    thr6 = np.stack([-t6hi, -t6lo]).astype(np.float16)   # [2, 4096]
    assert np.array_equal(thr6.astype(np.float64).sum(0), -t6u)

    w7r = np.zeros((16, 128, 2, 4096), np.float32)
    for kc in range(16):
        for j in range(2):
            w7r[kc, :, j, :] = w7[:, 256 * kc + 128 * j: 256 * kc + 128 * (j + 1)].T
    P['w7c'] = np.ascontiguousarray(
        w7r.reshape(16, 128, 2, 8, 512).transpose(3, 1, 0, 2, 4)
    ).reshape(8, 128, 16 * 1024).astype(FP8)

    s7, b7, m7 = bn_scale(inputs['bn7'])
    K7 = w7.sum(1)
    sc7 = (2.0 * s7).astype(np.float32).reshape(32, 128).T               # [128,32]
    bi7 = (((-K7 - m7) * s7) + b7).astype(np.float32).reshape(32, 128).T

    w8 = _f32(inputs['lin3_w'])
    w8t = np.zeros((32, 128, 1000), np.float32)
    for kc in range(32):
        w8t[kc] = w8[:, 128 * kc:128 * (kc + 1)].T
    P['w8c'] = np.ascontiguousarray(w8t.transpose(1, 0, 2)).reshape(
        128, 32 * 1000).astype(np.float16)
    b8 = _f32(inputs['lin3_b'])
    b8hi = b8.astype(np.float16).astype(np.float32)
    b8hl = np.stack([b8hi, b8 - b8hi]).astype(np.float16)                # [2,1000]

    gD = np.zeros((128, GD_LEN), np.float32)
    gD[0:96, GD_OFF['t1x']] = t1x
    gD[:, GD_OFF['thr2u']:GD_OFF['thr2u'] + 2] = thr2u
    gD[:, GD_OFF['thr3u']:GD_OFF['thr3u'] + 3] = thr3u
    gD[:, GD_OFF['thr4u']:GD_OFF['thr4u'] + 3] = thr4u
    gD[:, GD_OFF['thr5u']:GD_OFF['thr5u'] + 2] = thr5u
    gD[:, GD_OFF['sc7']:GD_OFF['sc7'] + 32] = sc7
    gD[:, GD_OFF['bi7']:GD_OFF['bi7'] + 32] = bi7
    gD[0:16, GD_OFF['id16f']:GD_OFF['id16f'] + 16] = np.eye(16, dtype=np.float32)
    P['gD'] = gD

    gE = np.zeros((2, GE_LEN), np.float16)
    gE[:, GE_OFF['thr6']:GE_OFF['thr6'] + 4096] = thr6
    gE[:, GE_OFF['b8']:GE_OFF['b8'] + 1000] = b8hl
    P['gE'] = gE

    P['id16'] = np.eye(16).astype(BF16)
    return P


IN_SPECS = [
    ('col_hi', (121, 3 * 3025), mybir.dt.float16, True),
    ('col_lo', (121, 3 * 3025), mybir.dt.float16, True),
    ('gA', (128, GA_LEN), mybir.dt.float8e4, False),
    ('gB', (96, GB_LEN), mybir.dt.float8e4, False),
    ('gC', (121, GC_LEN), mybir.dt.float16, False),
    ('gD', (128, GD_LEN), mybir.dt.float32, False),
    ('gE', (2, GE_LEN), mybir.dt.float16, False),
    ('w6c', (8, 128, 36 * 1024), mybir.dt.float8e4, False),
    ('w7c', (8, 128, 16 * 1024), mybir.dt.float8e4, False),
    ('w8c', (128, 32 * 1000), mybir.dt.float16, False),
    ('id16', (16, 16), mybir.dt.bfloat16, False),
]


def build_module(B, dbg=False):
    nc = bass.Bass("TRN2", target_bir_lowering=False, debug=False,
                   num_devices=NCORES)
    ins = {}
    for name, shp, dt, per_img in IN_SPECS:
        shape = (B,) + shp if per_img else shp
        ins[name] = nc.dram_tensor(name, list(shape), dt, kind="ExternalInput").ap()

    out = nc.dram_tensor("out", [B, 1000], mybir.dt.float32, kind="ExternalOutput").ap()
    dbg_outs = {}
    if dbg:
        for name, shape, dt in [
            ('d_u1', [B, 96, S2], mybir.dt.float8e4),
            ('d_u2', [B, 128, 2, S3], mybir.dt.float8e4),
            ('d_u3a', [B, 128, 2, S4], mybir.dt.float8e4),
            ('d_u3b', [B, 128, S4], mybir.dt.float8e4),
            ('d_u4a', [B, 128, 2, S4], mybir.dt.float8e4),
            ('d_u4b', [B, 128, S4], mybir.dt.float8e4),
            ('d_t5', [128, 2, 36, B], mybir.dt.float8e4),
            ('d_t6', [128, 16, 2, B], mybir.dt.float8e4),
        ]:
            dbg_outs[name] = nc.dram_tensor(name, shape, dt, kind="ExternalOutput").ap()

    dma_handles = []
    tail_extra = []

    def ldma(dst, src):        # im2col streaming queue (gpsimd)
        h = nc.gpsimd.dma_start(dst, src)
        dma_handles.append(h)
        return h

    def wdma(dst, src):        # weight streaming queue (sync/SP)
        h = nc.sync.dma_start(dst, src)
        dma_handles.append(h)
        return h

    def adma(dst, src):        # aux weight queue (scalar/ACT)
        h = nc.scalar.dma_start(dst, src)
        dma_handles.append(h)
        return h

    def odma(dst, src):
        h = nc.sync.dma_start(dst, src)
        dma_handles.append(h)
        return h

    with tile.TileContext(nc) as tc:
        with tc.tile_pool(name="wpool", bufs=1) as wp, \
             tc.tile_pool(name="cpool", bufs=1) as cp:

            # ---------- resident constant views ----------
            W = {}

            tD = wp.tile([128, GD_LEN], mybir.dt.float32, tag="gD")
            wdma(tD[:], ins['gD'][:])
            # fc-only constants (thr6/b8/id16) load on the aux queue so the
            # startup critical path is just gD -> conv weights -> first cols
            tE = wp.tile([2, GE_LEN], mybir.dt.float16, tag="gE")
            adma(tE[:], ins['gE'][:])
            tI = wp.tile([16, 16], mybir.dt.bfloat16, tag="id16")
            adma(tI[:], ins['id16'][:])
            W['id16'] = tI

            W['t1x'] = apv(tD[:], GD_OFF['t1x'], [[GD_LEN, 96], [1, 1]])
            for key, cnt in (('thr2u', 2), ('thr3u', 3), ('thr4u', 3),
                             ('thr5u', 2), ('sc7', 32), ('bi7', 32)):
                W[key] = [apv(tD[:], GD_OFF[key] + c, [[GD_LEN, 128], [1, 1]])
                          for c in range(cnt)]
            W['id16f'] = apv(tD[:], GD_OFF['id16f'], [[GD_LEN, 16], [1, 16]])
            W['thr6'] = [apv(tE[:], GE_OFF['thr6'] + n0, [[GE_LEN, 2], [1, 512]])
                         for n0 in range(0, 4096, 512)]
            W['b8'] = [apv(tE[:], GE_OFF['b8'] + h * 500, [[GE_LEN, 2], [1, 500]])
                       for h in range(2)]

            t_ones = wp.tile([2, 128], mybir.dt.float16, tag="ones")
            nc.vector.memset(t_ones[:], 1.0)

            t_zw = wp.tile([1, 128], mybir.dt.float8e4, tag="zw")
            nc.vector.memset(t_zw[:], 0.0)
            t_zr = wp.tile([1, 1024], mybir.dt.float8e4, tag="zr")
            nc.vector.memset(t_zr[:], 0.0)

            last_out = {}   # proc -> 1-cell AP of that engine's latest evac output

            # engine/queue intro: first-touch the packed-constant DMA queues
            scr = wp.tile([128, 1], mybir.dt.float32, tag="scr")
            nc.vector.tensor_copy(scr[:, 0:1], tD[:, 0:1])
            scrA = wp.tile([128, 1], mybir.dt.float32, tag="scrA")
            nc.scalar.copy(scrA[:, 0:1], tD[:, 0:1])
            scrE = wp.tile([2, 4], mybir.dt.float16, tag="scrE")
            nc.vector.tensor_copy(scrE[:, 0:1], tE[:, 0:1])
            scrEA = wp.tile([2, 4], mybir.dt.float16, tag="scrEA")
            nc.scalar.copy(scrEA[:, 0:1], tE[:, 0:1])

            # fc1 lhsT source, accumulated across all images
            Bp = max(16, B)   # fp8 DR slot strides must be 16B-aligned
            t5 = cp.tile([128, 2 * 36 * Bp], mybir.dt.float8e4, tag="t5")
            nc.vector.memset(t5[:], 0.0)

            # ================= conv phase =================
            with tc.tile_pool(name="cw", bufs=1) as cwp, \
                 tc.tile_pool(name="act", bufs=5) as ap_, \
                 tc.tile_pool(name="stage", bufs=1) as stp, \
                 tc.tile_pool(name="col", bufs=2) as colp, \
                 tc.tile_pool(name="ps1", bufs=2, space="PSUM") as ps1, \
                 tc.tile_pool(name="ps2", bufs=1, space="PSUM") as ps2, \
                 tc.tile_pool(name="ps3", bufs=4, space="PSUM") as ps3:

                # conv weights: three packed DMAs on the weight queue
                tC = cwp.tile([121, GC_LEN], mybir.dt.float16, tag="gC")
                wdma(tC[:], ins['gC'][:])
                tB = cwp.tile([96, GB_LEN], mybir.dt.float8e4, tag="gB")
                wdma(tB[:], ins['gB'][:])
                tA = cwp.tile([128, GA_LEN], mybir.dt.float8e4, tag="gA")
                wdma(tA[:], ins['gA'][:])

                for k, n in GC_ENTRIES:
                    W[k] = apv(tC[:], GC_OFF[k], [[GC_LEN, 121], [1, 96]])
                for k, n in GB_ENTRIES:
                    if n == 256:
                        W[k] = apv(tB[:], GB_OFF[k], [[GB_LEN, 96], [128, 2], [1, 128]])
                    else:
                        W[k] = apv(tB[:], GB_OFF[k], [[GB_LEN, 96], [1, 128]])
                for k, n in GA_ENTRIES:
                    if n == 256:
                        W[k] = apv(tA[:], GA_OFF[k], [[GA_LEN, 128], [128, 2], [1, 128]])
                    else:
                        W[k] = apv(tA[:], GA_OFF[k], [[GA_LEN, 128], [1, 128]])

                NB = 4
                for i0 in range(0, B, NB):
                    blk = list(range(i0, min(B, i0 + NB)))
                    u2s, u3as, u3bs, u4as, u4bs = {}, {}, {}, {}, {}
                    for i in blk:
                        # ---- conv1 ----
                        colh = colp.tile([121, 3 * 3025], mybir.dt.float16, tag="colh")
                        ldma(colh[:], ins['col_hi'][i])
                        coll = colp.tile([121, 3 * 3025], mybir.dt.float16, tag="coll")
                        ldma(coll[:], ins['col_lo'][i])

                        c1f = stp.tile([96, 3025], mybir.dt.float32, tag="c1f")
                        passes = [('w1_hi', colh), ('w1_hi', coll), ('w1_lo', colh)]
                        for n0 in range(0, 3025, 512):
                            n1 = min(3025, n0 + 512)
                            pc = ps1.tile([96, 512], mybir.dt.float32, tag="pc1")
                            cnt = 0
                            for ct in range(3):
                                for wk, colt in passes:
                                    cnt += 1
                                    nc.tensor.matmul(
                                        pc[:, 0:n1 - n0], W[wk + str(ct)],
                                        apv(colt[:], ct * 3025 + n0,
                                            [[3 * 3025, 121], [1, n1 - n0]]),
                                        start=(cnt == 1), stop=(cnt == 9))
                            last_out['ACT'] = nc.scalar.copy(c1f[:, n0:n1], pc[:, 0:n1 - n0])

                        # pool1: 55x55 -> 27x27
                        pm1 = stp.tile([96, 55 * 27], mybir.dt.float32, tag="pm1")
                        d = [[3025, 96], [55, 55], [2, 27]]
                        o = [[55 * 27, 96], [27, 55], [1, 27]]
                        nc.vector.tensor_max(apv(pm1[:], 0, o), apv(c1f[:], 0, d), apv(c1f[:], 1, d))
                        nc.vector.tensor_max(apv(pm1[:], 0, o), apv(pm1[:], 0, o), apv(c1f[:], 2, d))
                        po1 = stp.tile([96, 729], mybir.dt.float32, tag="po1")
                        d2 = [[55 * 27, 96], [54, 27], [1, 27]]
                        o2 = [[729, 96], [27, 27], [1, 27]]
                        nc.vector.tensor_max(apv(po1[:], 0, o2), apv(pm1[:], 0, d2), apv(pm1[:], 27, d2))
                        nc.vector.tensor_max(apv(po1[:], 0, o2), apv(po1[:], 0, o2), apv(pm1[:], 54, d2))

                        u1 = ap_.tile([96, S2], mybir.dt.float8e4, tag="u1")
                        nc.vector.memset(u1[:], 0.0)
                        last_out['DVE'] = nc.vector.tensor_scalar(
                            apv(u1[:], B2, [[S2, 96], [G2, 27], [1, 27]]),
                            apv(po1[:], 0, o2), W['t1x'], None,
                            mybir.AluOpType.is_ge)
                        if dbg:
                            odma(dbg_outs['d_u1'][i], u1[:])

                        # ---- conv2 ----
                        u2 = ap_.tile([128, 2 * S3], mybir.dt.float8e4, tag="u2")
                        nc.vector.memset(u2[:], 0.0)
                        for mt in range(2):
                            p2 = ps2.tile([128, N2], mybir.dt.float32, tag="p2")
                            nmm = 0
                            tot = 5 * 3 * 2
                            for kx in range(5):
                                dx = kx - 2
                                for kg in range(2):
                                    off = B2 + (2 * kg - 2) * G2 + dx
                                    for n0, n1 in ((0, 512), (512, 864)):
                                        nmm += 1
                                        nc.tensor.matmul(
                                            p2[:, n0:n1], W[f'w2dr{mt}{kx}{kg}'],
                                            apv(u1[:], off + n0,
                                                [[S2, 96], [G2, 2], [1, n1 - n0]]),
                                            start=(kx == 0 and kg == 0),
                                            stop=(nmm >= tot - 1), perf_mode=DR)
                                off = B2 + 2 * G2 + dx
                                for n0, n1 in ((0, 512), (512, 864)):
                                    nmm += 1
                                    # ky=4 paired with an all-zero phantom row
                                    nc.tensor.matmul(
                                        p2[:, n0:n1], W[f'w2sr{mt}{kx}'],
                                        apv(u1[:], off + n0, [[S2, 96], [G2, 2], [1, n1 - n0]]),
                                        start=False, stop=(nmm >= tot - 1),
                                        perf_mode=DR)
                            c2f = stp.tile([128, N2], mybir.dt.float32, tag="c2f")
                            last_out['ACT'] = nc.scalar.copy(c2f[:], p2[:])
                            pm2 = stp.tile([128, 27 * 13], mybir.dt.float32, tag="pm2")
                            d = [[N2, 128], [G2, 27], [2, 13]]
                            o = [[27 * 13, 128], [13, 27], [1, 13]]
                            nc.vector.tensor_max(apv(pm2[:], 0, o), apv(c2f[:], 0, d), apv(c2f[:], 1, d))
                            nc.vector.tensor_max(apv(pm2[:], 0, o), apv(pm2[:], 0, o), apv(c2f[:], 2, d))
                            po2 = stp.tile([128, 169], mybir.dt.float32, tag="po2")
                            d2 = [[27 * 13, 128], [26, 13], [1, 13]]
                            o2 = [[169, 128], [13, 13], [1, 13]]
                            nc.vector.tensor_max(apv(po2[:], 0, o2), apv(pm2[:], 0, d2), apv(pm2[:], 13, d2))
                            nc.vector.tensor_max(apv(po2[:], 0, o2), apv(po2[:], 0, o2), apv(pm2[:], 26, d2))
                            last_out['DVE'] = nc.vector.tensor_scalar(
                                apv(u2[:], mt * S3 + B3, [[2 * S3, 128], [G3, 13], [1, 13]]),
                                apv(po2[:], 0, o2), W['thr2u'][mt], None,
                                mybir.AluOpType.is_ge)
                        u2s[i] = u2
                        if dbg:
                            odma(dbg_outs['d_u2'][i], u2[:].rearrange("p (a b) -> p a b", a=2))

                    # ---- conv3: image-blocked (shared weight loads) ----
                    for i in blk:
                        u3a = ap_.tile([128, 2 * S4], mybir.dt.float8e4, tag="u3a")
                        nc.vector.memset(u3a[:], 0.0)
                        u3b = ap_.tile([128, S4], mybir.dt.float8e4, tag="u3b")
                        nc.vector.memset(u3b[:], 0.0)
                        u3as[i], u3bs[i] = u3a, u3b
                    for mt in range(3):
                        p3s = {}
                        for i in blk:
                            p3f = ps3.tile([128, 512], mybir.dt.float32, tag="p45")
                            p3s[i] = p3f[:, 0:N3]
                        for ky in range(3):
                            for kx in range(3):
                                off = B3 + (ky - 1) * G3 + (kx - 1)
                                for i in blk:
                                    nc.tensor.matmul(
                                        p3s[i], W[f'w3dr{mt}{ky}{kx}'],
                                        apv(u2s[i][:], off, [[2 * S3, 128], [S3, 2], [1, N3]]),
                                        start=(ky == 0 and kx == 0),
                                        stop=(ky == 2 and kx == 2),
                                        perf_mode=DR)
                        for i in blk:
                            srcp = apv(p3s[i], 0, [[512, 128], [G3, 13], [1, 13]])
                            if mt < 2:
                                dstu = apv(u3as[i][:], mt * S4 + B4, [[2 * S4, 128], [G4, 13], [1, 13]])
                            else:
                                dstu = apv(u3bs[i][:], B4, [[S4, 128], [G4, 13], [1, 13]])
                            last_out['DVE'] = nc.vector.tensor_scalar(
                                dstu, srcp, W['thr3u'][mt],
                                None, mybir.AluOpType.is_ge)
                    if dbg:
                        for i in blk:
                            odma(dbg_outs['d_u3a'][i], u3as[i][:].rearrange("p (a b) -> p a b", a=2))
                            odma(dbg_outs['d_u3b'][i], u3bs[i][:])

                    # ---- conv4 / conv5: image-blocked ----
                    for i in blk:
                        u4a = ap_.tile([128, 2 * S4], mybir.dt.float8e4, tag="u4a")
                        nc.vector.memset(u4a[:], 0.0)
                        u4b = ap_.tile([128, S4], mybir.dt.float8e4, tag="u4b")
                        nc.vector.memset(u4b[:], 0.0)
                        u4as[i], u4bs[i] = u4a, u4b

                    def conv45blk(nm, nmt, uas, ubs, sink):
                        for mt in range(nmt):
                            p4s = {}
                            for i in blk:
                                p4f = ps3.tile([128, 512], mybir.dt.float32, tag="p45")
                                p4s[i] = p4f[:, 0:N4]
                            for ky in range(3):
                                for kx in range(3):
                                    off = B4 + (ky - 1) * G4 + (kx - 1)
                                    for i in blk:
                                        nc.tensor.matmul(
                                            p4s[i], W[f'{nm}a{mt}{ky}{kx}'],
                                            apv(uas[i][:], off, [[2 * S4, 128], [S4, 2], [1, N4]]),
                                            start=(ky == 0 and kx == 0),
                                            stop=False, perf_mode=DR)
                            for kx in range(3):
                                dx = kx - 1
                                for i in blk:
                                    nc.tensor.matmul(
                                        p4s[i], W[f'{nm}bd{mt}{kx}'],
                                        apv(ubs[i][:], B4 - G4 + dx, [[S4, 128], [G4, 2], [1, N4]]),
                                        start=False, stop=False, perf_mode=DR)
                            for kx in range(3):
                                dx = kx - 1
                                for i in blk:
                                    # ky=2 paired with an all-zero phantom row
                                    nc.tensor.matmul(
                                        p4s[i], W[f'{nm}bs{mt}{kx}'],
                                        apv(ubs[i][:], B4 + G4 + dx, [[S4, 128], [G4, 2], [1, N4]]),
                                        start=False, stop=(kx == 2), perf_mode=DR)
                            for i in blk:
                                sink(mt, i, p4s[i])

                    def sink4(mt, i, p4):
                        srcp = apv(p4, 0, [[512, 128], [G4, 13], [1, 13]])
                        if mt < 2:
                            dstu = apv(u4as[i][:], mt * S4 + B4, [[2 * S4, 128], [G4, 13], [1, 13]])
                        else:
                            dstu = apv(u4bs[i][:], B4, [[S4, 128], [G4, 13], [1, 13]])
                        last_out['DVE'] = nc.vector.tensor_scalar(
                            dstu, srcp, W['thr4u'][mt],
                            None, mybir.AluOpType.is_ge)

                    conv45blk('w4', 3, u3as, u3bs, sink4)
                    if dbg:
                        for i in blk:
                            odma(dbg_outs['d_u4a'][i], u4as[i][:].rearrange("p (a b) -> p a b", a=2))
                            odma(dbg_outs['d_u4b'][i], u4bs[i][:])

                    def sink5(mt, i, p5):
                        c5f = stp.tile([128, N4], mybir.dt.float32, tag="c5f")
                        last_out['ACT'] = nc.scalar.copy(c5f[:], p5)
                        pm5 = stp.tile([128, 13 * 6], mybir.dt.float32, tag="pm5")
                        d = [[N4, 128], [G4, 13], [2, 6]]
                        o = [[13 * 6, 128], [6, 13], [1, 6]]
                        nc.vector.tensor_max(apv(pm5[:], 0, o), apv(c5f[:], 0, d), apv(c5f[:], 1, d))
                        nc.vector.tensor_max(apv(pm5[:], 0, o), apv(pm5[:], 0, o), apv(c5f[:], 2, d))
                        po5 = stp.tile([128, 36], mybir.dt.float32, tag="po5")
                        d2 = [[13 * 6, 128], [12, 6], [1, 6]]
                        o2 = [[36, 128], [6, 6], [1, 6]]
                        nc.vector.tensor_max(apv(po5[:], 0, o2), apv(pm5[:], 0, d2), apv(pm5[:], 6, d2))
                        nc.vector.tensor_max(apv(po5[:], 0, o2), apv(po5[:], 0, o2), apv(pm5[:], 12, d2))
                        h5 = nc.vector.tensor_scalar(
                            apv(t5[:], mt * 36 * Bp + i, [[2 * 36 * Bp, 128], [Bp, 36]]),
                            po5[:], W['thr5u'][mt], None,
                            mybir.AluOpType.is_ge)
                        last_out['DVE'] = h5
                        if i == B - 1:
                            tail_extra.append(h5)

                    conv45blk('w5', 2, u4as, u4bs, sink5)

            if dbg:
                odma(dbg_outs['d_t5'][:],
                     t5[:].rearrange("p (a b c) -> p a b c", a=2, b=36)
                     if Bp == B else
                     apv(t5[:], 0, [[2 * 36 * Bp, 128], [36 * Bp, 2], [Bp, 36], [1, B]]))

            # ================= fc phase =================
            with tc.tile_pool(name="fcw", bufs=2) as fcw, \
                 tc.tile_pool(name="fc8", bufs=1) as fc8p, \
                 tc.tile_pool(name="psf", bufs=2, space="PSUM") as psf, \
                 tc.tile_pool(name="pst", bufs=2, space="PSUM") as pst, \
                 tc.tile_pool(name="ps3f", bufs=1, space="PSUM") as ps3f:

                # fc3 weights stream on the aux queue while fc1 runs
                # (piece-split and dep-interleaved variants both simulated
                # slower; the scheduler does best with one transfer here)
                w8t_t = fc8p.tile([128, 32 * 1000], mybir.dt.float16, tag="w8c")
                adma(w8t_t[:], ins['w8c'][:])

                # fc1 (by output chunks of 512); weights as one contiguous
                # chunk DMA each, double-buffered on the weight queue
                t6b = []
                for n in range(8):
                    t6b_n = cp.tile([B, 512], mybir.dt.bfloat16, tag=f"t6b{n}")
                    t6b.append(t6b_n)
                for nchi in range(8):
                    wc = fcw.tile([128, 36 * 1024], mybir.dt.float8e4, tag="w6c")
                    # split each chunk across the two idle DMA queues
                    wdma(apv(wc[:], 0, [[36 * 1024, 128], [1, 18 * 1024]]),
                         apv(ins['w6c'][nchi], 0, [[36 * 1024, 128], [1, 18 * 1024]]))
                    ldma(apv(wc[:], 18 * 1024, [[36 * 1024, 128], [1, 18 * 1024]]),
                         apv(ins['w6c'][nchi], 18 * 1024, [[36 * 1024, 128], [1, 18 * 1024]]))
                    pf = psf.tile([B, 512], mybir.dt.float32, tag="pf")
                    for s in range(36):
                        nc.tensor.matmul(
                            pf[:], apv(t5[:], s * Bp, [[2 * 36 * Bp, 128], [36 * Bp, 2], [1, B]]),
                            apv(wc[:], s * 1024, [[36 * 1024, 128], [512, 2], [1, 512]]),
                            start=(s == 0), stop=False, perf_mode=DR)
                    nc.tensor.matmul(pf[:], t_ones[:, 0:B], W['thr6'][nchi],
                                     start=False, stop=True)
                    last_out['DVE'] = nc.vector.tensor_scalar(
                        t6b[nchi][:], pf[:], 0.0, None, mybir.AluOpType.is_ge)

                # transpose to t6[kc] tiles [128, (j, B)]
                t6 = []
                for kc in range(16):
                    t6_kc = cp.tile([128, 2 * Bp], mybir.dt.float8e4, tag=f"t6_{kc}")
                    t6.append(t6_kc)
                Ba = ((B + 1) // 2) * 2   # 4-byte-aligned slot stride for bf16
                for kc in range(16):
                    pt2f = pst.tile([128, 1024], mybir.dt.bfloat16, tag="ptr")
                    for j in range(2):
                        src_col = 256 * kc + 128 * j
                        nc.tensor.transpose(
                            pt2f[:, j * Ba:j * Ba + B],
                            t6b[src_col // 512][:, src_col % 512: src_col % 512 + 128],
                            W['id16'][0:B, 0:B])
                    last_out['ACT'] = nc.scalar.copy(
                        apv(t6[kc][:], 0, [[2 * Bp, 128], [Bp, 2], [1, B]]),
                        apv(pt2f[:], 0, [[1024, 128], [Ba, 2], [1, B]]))
                if dbg:
                    for kc in range(16):
                        odma(apv(dbg_outs['d_t6'][:], kc * 2 * B,
                                 [[16 * 2 * B, 128], [B, 2], [1, B]]),
                             apv(t6[kc][:], 0, [[2 * Bp, 128], [Bp, 2], [1, B]]))

                # fc2
                s7f = []
                for n in range(8):
                    s7f_n = cp.tile([B, 512], mybir.dt.float32, tag=f"s7f{n}")
                    s7f.append(s7f_n)
                for nchi in range(8):
                    # reuse the fc1 chunk ring (same tag/shape): fc2 chunks
                    # occupy the first 16K elements of a w6c-shaped tile
                    wc = fcw.tile([128, 36 * 1024], mybir.dt.float8e4, tag="w6c")
                    wdma(apv(wc[:], 0, [[36 * 1024, 128], [1, 8 * 1024]]),
                         apv(ins['w7c'][nchi], 0, [[16 * 1024, 128], [1, 8 * 1024]]))
                    ldma(apv(wc[:], 8 * 1024, [[36 * 1024, 128], [1, 8 * 1024]]),
                         apv(ins['w7c'][nchi], 8 * 1024, [[16 * 1024, 128], [1, 8 * 1024]]))
                    pf = psf.tile([B, 512], mybir.dt.float32, tag="pf")
                    for kc in range(16):
                        nc.tensor.matmul(
                            pf[:], apv(t6[kc][:], 0, [[2 * Bp, 128], [Bp, 2], [1, B]]),
                            apv(wc[:], kc * 1024, [[36 * 1024, 128], [512, 2], [1, 512]]),
                            start=(kc == 0), stop=(kc == 15), perf_mode=DR)
                    last_out['DVE'] = nc.vector.tensor_copy(s7f[nchi][:], pf[:])

                # transpose + bn7-relu -> y7 [128, (kc, B)] fp16
                y7 = []
                for kc in range(32):
                    y7_kc = cp.tile([128, B], mybir.dt.float16, tag=f"y7_{kc}")
                    y7.append(y7_kc)
                for kc in range(32):
                    ptf = pst.tile([128, 512], mybir.dt.float32, tag="ptrf")
                    pt = ptf[:, 0:B]
                    col = 128 * kc
                    nc.tensor.transpose(pt, s7f[col // 512][:, col % 512: col % 512 + 128],
                                        W['id16f'][0:B, 0:B])
                    hact = nc.scalar.activation(
                        y7[kc][:], pt,
                        mybir.ActivationFunctionType.Relu,
                        bias=W['bi7'][kc], scale=W['sc7'][kc])
                    last_out['ACT'] = hact

                # fc3
                pf3af = ps3f.tile([B, 512], mybir.dt.float32, tag="pf3a")
                pf3a = pf3af[:, 0:500]
                pf3bf = ps3f.tile([B, 512], mybir.dt.float32, tag="pf3b")
                pf3b = pf3bf[:, 0:500]
                for kc in range(32):
                    lhs = y7[kc][:]
                    nc.tensor.matmul(pf3a, lhs,
                                     apv(w8t_t[:], kc * 1000, [[32 * 1000, 128], [1, 500]]),
                                     start=(kc == 0), stop=False)
                    nc.tensor.matmul(pf3b, lhs,
                                     apv(w8t_t[:], kc * 1000 + 500, [[32 * 1000, 128], [1, 500]]),
                                     start=(kc == 0), stop=False)
                nc.tensor.matmul(pf3a, t_ones[:, 0:B], W['b8'][0],
                                 start=False, stop=True)
                nc.tensor.matmul(pf3b, t_ones[:, 0:B], W['b8'][1],
                                 start=False, stop=True)

                of = cp.tile([B, 1000], mybir.dt.float32, tag="of")
                nc.vector.tensor_copy(of[:, 0:500], pf3a)
                h_of = nc.vector.tensor_copy(of[:, 500:1000], pf3b)
                tail_extra.append(h_of)
                dma_handles.append(nc.sync.dma_start(out[:], of[:]))

            # ---------- tail-sync for the final drain ----------
            for h in dma_handles[-40:] + tail_extra:
                n = nc.sync.nop(nofuse=True)
                add_dep_helper(n.ins, h.ins, reason="tail drain sync")

    legalize_waits(nc)
    return nc


def legalize_waits(nc):
    """Split multi-wait sync lists into single-wait same-engine NOPs.

    TPB instructions (compute, NOP, drain, DMA pseudo-ops) accept one
    sync-wait command in this walrus; extra waits are moved onto freshly
    inserted NOPs placed directly before the instruction in its basic block
    (same engine stream).
    """
    f = nc.m.functions[0]
    ctr = 0
    ndedup = 0
    for blk in f.blocks:
        new = []
        last_lw_key = None
        for inst in blk.instructions:
            tname0 = type(inst).__name__
            if tname0 == 'InstLdweights':
                si0 = inst.sync_info
                key = (str(inst.ins[0]), str(inst.perf_mode))
                if False and key == last_lw_key and not (si0 and si0.on_wait):
                    ndedup += 1
                    continue          # redundant reload of identical weights
                last_lw_key = key
            si = inst.sync_info
            tname = type(inst).__name__
            if si is not None and inst.engine is not None:
                waits = list(si.on_wait)
                if len(waits) > 1:
                    for w in waits[:-1]:
                        ctr += 1
                        n = mybir.InstNoOp(name=f"I-wfix{ctr}", ins=[], outs=[])
                        n.engine = inst.engine
                        n.sync_info = bass_rust.SyncInfo(on_wait=[w], on_update=[])
                        new.append(n)
                    inst.sync_info = bass_rust.SyncInfo(
                        on_wait=[waits[-1]], on_update=list(si.on_update))
            new.append(inst)
        blk.instructions = new
    return ctr, ndedup


# ======================= entry point =======================

def make_in_maps(P, B):
    in_maps = []
    for c in range(NCORES):
        m = {}
        for name, shp, dt, per_img in IN_SPECS:
            a = P[name]
            if per_img:
                a = a[c * B:(c + 1) * B]
            m[name] = np.ascontiguousarray(a)
        in_maps.append(m)
    return in_maps


def kernel(**inputs) -> np.ndarray:
    P = prep_host(inputs)
    B = P['col_hi'].shape[0] // NCORES
    nc = build_module(B, dbg=False)
    in_maps = make_in_maps(P, B)
    res = run_bass_kernel_spmd(nc, in_maps, core_ids=list(range(NCORES)))
    outs = [res.results[c]['out'] for c in range(NCORES)]
    return np.concatenate(outs, axis=0).astype(np.float32)


# revision 33
# speedup vs baseline: 1.2551x; 1.2551x over previous
"""Binarized AlexNet on 8 Trainium2 NeuronCores (SPMD, data-parallel over batch).

- Batch 128 sharded 16 imgs/core; weights replicated.
- conv1: 3-pass fp16 (hi/lo split) matmuls over host im2col (error ~1e-6 so the
  bn1 sign decisions match the fp32 reference; bn1 boundary found by bit-exact
  fp32 emulation).
- Binarized activations kept as u in {0,1} fp8: pad(-1) maps to u=0, so conv
  inputs live on zero-padded flat grids; DoubleRow slots are row-shifts /
  channel-group interleaves; sums are exact integers in fp32 PSUM.
- Binarize decisions via host-precomputed per-channel integer thresholds that
  exactly emulate the reference's fp32 bn rounding (validated: 0 mismatches
  over all (channel, attainable-integer) pairs). Half-offset -> never ties.
- fc1/fc2: activation-stationary fp8 DR matmuls; thresholds via rank-1 (K=2
  fp16) matmuls; PE transposes between layers; bn7 folded into ACT Relu; fc3
  in fp16.
- DMA strategy: resident weights packed host-side into 5 group blobs (one DMA
  each); im2col tiles are single contiguous DMAs; fc1/fc2 weights stream as
  8 chunk DMAs each (contiguous 4.7MB / 2.1MB) double-buffered, fc3 as one
  8MB DMA, spread across the gpsimd/sync/scalar DMA queues so weight
  streaming overlaps compute instead of bottlenecking on descriptor issue.
"""
import dataclasses
import numpy as np
import ml_dtypes

import bass_rust
import concourse.bass as bass
import concourse.tile as tile
from concourse import mybir
from concourse.bass_utils import run_bass_kernel_spmd
from concourse.tile_rust import add_dep_helper

FP8 = ml_dtypes.float8_e4m3
BF16 = ml_dtypes.bfloat16

EPS = np.float32(1e-5)
NCORES = 8
DR = mybir.MatmulPerfMode.DoubleRow

# grids: (stride, base, tile-size, out-flat-count)
# S2/S4 carry extra zero rows so the all-zero phantom DoubleRow slot of the
# last conv2 / conv4/5 tap row stays in-bounds (reads zeros, weight is 0).
G2, B2, S2, N2 = 32, 66, 1152, 27 * 32          # conv2 in-grid (27x27 interior)
G3, B3, S3, N3 = 15, 16, 240, 13 * 15           # conv3 in-grid (13x13)
G4, B4, S4, N4 = 16, 17, 272, 13 * 16           # conv4/5 in-grid (13x13)


def apv(base, offset_add, dims):
    return dataclasses.replace(base, offset=base.offset + offset_add, ap=dims)


# ---- packed resident-weight group layouts (host layout == SBUF layout) ----

def _layout(entries):
    off, o = {}, 0
    for k, n in entries:
        off[k] = o
        o += n
    return off, o


GA_ENTRIES = (
    [(f'w3dr{mt}{ky}{kx}', 256) for mt in range(3) for ky in range(3) for kx in range(3)]
    + [(f'w4a{mt}{ky}{kx}', 256) for mt in range(3) for ky in range(3) for kx in range(3)]
    + [(f'w4bd{mt}{kx}', 256) for mt in range(3) for kx in range(3)]
    + [(f'w4bs{mt}{kx}', 256) for mt in range(3) for kx in range(3)]
    + [(f'w5a{mt}{ky}{kx}', 256) for mt in range(2) for ky in range(3) for kx in range(3)]
    + [(f'w5bd{mt}{kx}', 256) for mt in range(2) for kx in range(3)]
    + [(f'w5bs{mt}{kx}', 256) for mt in range(2) for kx in range(3)])
GA_OFF, GA_LEN = _layout(GA_ENTRIES)            # fp8, 128 rows
GB_ENTRIES = (
    [(f'w2dr{mt}{kx}{kg}', 256) for mt in range(2) for kx in range(5) for kg in range(2)]
    + [(f'w2sr{mt}{kx}', 256) for mt in range(2) for kx in range(5)])
GB_OFF, GB_LEN = _layout(GB_ENTRIES)            # fp8, 96 rows
GC_ENTRIES = ([(f'w1_hi{ct}', 96) for ct in range(3)]
              + [(f'w1_lo{ct}', 96) for ct in range(3)])
GC_OFF, GC_LEN = _layout(GC_ENTRIES)            # fp16, 121 rows
GD_ENTRIES = [('t1x', 1), ('thr2u', 2), ('thr3u', 3), ('thr4u', 3), ('thr5u', 2),
              ('sc7', 32), ('bi7', 32), ('id16f', 16)]
GD_OFF, GD_LEN = _layout(GD_ENTRIES)            # f32, 128 rows
GE_ENTRIES = [('thr6', 4096), ('b8', 1000)]
GE_OFF, GE_LEN = _layout(GE_ENTRIES)            # f16, 2 rows


# ======================= host-side preparation =======================

def _f32(x):
    return np.asarray(x, dtype=np.float32)


def bn_scale(bnp):
    g, b, m, v = [_f32(a) for a in bnp]
    s = (g / np.sqrt(v + EPS)).astype(np.float32)
    return s, b, m


def exact_bn1_threshold(bnp):
    """Per-channel minimal fp32 x with ((x-m)*s)+b >= 0 under fp32 rounding."""
    s, b, m = bn_scale(bnp)
    assert np.all(s > 0)
    C = len(s)

    def dec(x):
        with np.errstate(over='ignore', invalid='ignore'):
            return ((x - m).astype(np.float32) * s).astype(np.float32) + b >= 0

    def key(f):  # monotone uint32 encoding of fp32 order
        i = f.view(np.uint32).astype(np.uint32)
        neg = (i >> 31).astype(bool)
        return np.where(neg, ~i, i | np.uint32(0x80000000)).astype(np.uint32)

    def unkey(k):
        k = k.astype(np.uint32)
        hi = (k >> 31).astype(bool)
        return np.where(hi, k & np.uint32(0x7FFFFFFF), ~k).astype(np.uint32).view(np.float32)

    lo = np.full(C, -3.0e38, np.float32)
    hi = np.full(C, 3.0e38, np.float32)
    always = dec(lo)
    never = ~dec(hi)
    klo, khi = key(lo).astype(np.uint64), key(hi).astype(np.uint64)
    for _ in range(40):
        kmid = ((klo + khi) // 2).astype(np.uint64)
        d = dec(unkey(kmid.astype(np.uint32)))
        khi = np.where(d, kmid, khi)
        klo = np.where(d, klo, kmid)
    t = unkey(khi.astype(np.uint32))
    t = np.where(always, np.float32(-3.0e38), t)
    t = np.where(never, np.float32(3.0e38), t)
    chk = dec(t) | never
    assert np.all(chk), "bn1 threshold search failed"
    return t.astype(np.float32)


def integer_threshold(bnp, rmax):
    """Min even integer E in [-rmax,rmax] with fp32-bn(E) >= 0, per channel."""
    s, b, m = bn_scale(bnp)
    assert np.all(s > 0)
    C = len(s)
    T = np.zeros(C, np.float64)
    E = np.arange(-rmax, rmax + 1, 2, dtype=np.float32)
    for c0 in range(0, C, 512):
        c1 = min(C, c0 + 512)
        pre = ((E[None, :] - m[c0:c1, None]).astype(np.float32)
               * s[c0:c1, None]).astype(np.float32) + b[c0:c1, None]
        d = pre >= 0
        anyd = d.any(axis=1)
        t = E[np.argmax(d, axis=1)].astype(np.float64)
        t = np.where(anyd, t, rmax + 2)
        t = np.where(d[:, 0], -rmax - 2, t)
        T[c0:c1] = t
    return T


def binarize_w(w):
    return np.where(_f32(w) >= 0, 1.0, -1.0).astype(np.float32)


def prep_host(inputs):
    P = {}
    x = _f32(inputs['x'])
    Bfull = x.shape[0]

    # ---- conv1 im2col (fp16 hi/lo), [B, 121, 3*3025] so one DMA per tile ----
    xp = np.zeros((Bfull, 3, 228, 228), np.float32)
    xp[:, :, 2:226, 2:226] = x
    xh = xp.astype(np.float16)
    xl = (xp - xh.astype(np.float32)).astype(np.float16)

    def im2col(a):
        s = a.strides
        v = np.lib.stride_tricks.as_strided(
            a, (Bfull, 3, 11, 11, 55, 55),
            (s[0], s[1], s[2], s[3], s[2] * 4, s[3] * 4))
        v = v.reshape(Bfull, 3, 121, 3025).transpose(0, 2, 1, 3)
        return np.ascontiguousarray(v).reshape(Bfull, 121, 3 * 3025)

    P['col_hi'] = im2col(xh)
    P['col_lo'] = im2col(xl)

    w1 = _f32(inputs['conv1_w']).reshape(96, 3, 121)
    w1h = w1.astype(np.float16)
    w1l = (w1 - w1h.astype(np.float32)).astype(np.float16)
    w1ht = w1h.transpose(1, 2, 0)                # [3,121,96]
    w1lt = w1l.transpose(1, 2, 0)

    gC = np.zeros((121, GC_LEN), np.float16)
    for ct in range(3):
        gC[:, GC_OFF[f'w1_hi{ct}']:GC_OFF[f'w1_hi{ct}'] + 96] = w1ht[ct]
        gC[:, GC_OFF[f'w1_lo{ct}']:GC_OFF[f'w1_lo{ct}'] + 96] = w1lt[ct]
    P['gC'] = gC

    t1x = exact_bn1_threshold(inputs['bn1'])     # [96]

    w2 = binarize_w(inputs['bconv2_w'])
    w3 = binarize_w(inputs['bconv3_w'])
    w4 = binarize_w(inputs['bconv4_w'])
    w5 = binarize_w(inputs['bconv5_w'])
    w6 = binarize_w(inputs['blin1_w'])
    w7 = binarize_w(inputs['blin2_w'])

    gB = np.zeros((96, GB_LEN), np.float32)
    for mt in range(2):
        for kx in range(5):
            for kg in range(2):
                blk = np.zeros((96, 2, 128), np.float32)
                for j in range(2):
                    blk[:, j, :] = w2[mt * 128:(mt + 1) * 128, :, 2 * kg + j, kx].T
                o = GB_OFF[f'w2dr{mt}{kx}{kg}']
                gB[:, o:o + 256] = blk.reshape(96, 256)
            o = GB_OFF[f'w2sr{mt}{kx}']
            gB[:, o:o + 128] = w2[mt * 128:(mt + 1) * 128, :, 4, kx].T
    P['gB'] = gB.astype(FP8)

    K2 = w2.reshape(256, -1).sum(1)
    T2 = integer_threshold(inputs['bn2'], 2400)
    thr2u = ((T2 + K2) / 2 - 0.5).astype(np.float32).reshape(2, 128).T  # [128,2]

    gA = np.zeros((128, GA_LEN), np.float32)

    def put_dr(key, w, mt, ky, kx):
        blk = np.zeros((128, 2, 128), np.float32)
        for j in range(2):
            blk[:, j, :] = w[mt * 128:(mt + 1) * 128, 128 * j:128 * (j + 1), ky, kx].T
        o = GA_OFF[key]
        gA[:, o:o + 256] = blk.reshape(128, 256)

    for mt in range(3):
        for ky in range(3):
            for kx in range(3):
                put_dr(f'w3dr{mt}{ky}{kx}', w3, mt, ky, kx)

    K3 = w3.reshape(384, -1).sum(1)
    T3 = integer_threshold(inputs['bn3'], 2304)
    thr3u = ((T3 + K3) / 2 - 0.5).astype(np.float32).reshape(3, 128).T  # [128,3]

    def pack45(nm, w, nmt):
        for mt in range(nmt):
            for kx in range(3):
                for ky in range(3):
                    put_dr(f'{nm}a{mt}{ky}{kx}', w, mt, ky, kx)
                blk = np.zeros((128, 2, 128), np.float32)
                for j in range(2):
                    blk[:, j, :] = w[mt * 128:(mt + 1) * 128, 256:384, j, kx].T
                o = GA_OFF[f'{nm}bd{mt}{kx}']
                gA[:, o:o + 256] = blk.reshape(128, 256)
                o = GA_OFF[f'{nm}bs{mt}{kx}']
                gA[:, o:o + 128] = w[mt * 128:(mt + 1) * 128, 256:384, 2, kx].T

    pack45('w4', w4, 3)
    K4 = w4.reshape(384, -1).sum(1)
    T4 = integer_threshold(inputs['bn4'], 3456)
    thr4u = ((T4 + K4) / 2 - 0.5).astype(np.float32).reshape(3, 128).T

    pack45('w5', w5, 2)
    K5 = w5.reshape(256, -1).sum(1)
    T5 = integer_threshold(inputs['bn5'], 3456)
    thr5u = ((T5 + K5) / 2 - 0.5).astype(np.float32).reshape(2, 128).T

    P['gA'] = gA.astype(FP8)

    # fc1 rhs: w6r[s][p, j, n] = w6[n, (p+128j)*36 + s], chunked contiguous:
    # w6c[c, p, s*1024 + j*512 + nn] = w6r[s, p, j, 512c + nn]
    w6r = np.zeros((36, 128, 2, 4096), np.float32)
    for j in range(2):
        for s in range(36):
            cols = (np.arange(128) + 128 * j) * 36 + s
            w6r[s, :, j, :] = w6[:, cols].T
    P['w6c'] = np.ascontiguousarray(
        w6r.reshape(36, 128, 2, 8, 512).transpose(3, 1, 0, 2, 4)
    ).reshape(8, 128, 36 * 1024).astype(FP8)
    K6 = w6.sum(1)
    T6 = integer_threshold(inputs['bn6'], 9216)
    t6u = (T6 + K6) / 2 - 0.5
    t6hi = np.floor(t6u / 16) * 16
    t6lo = t6u - t6hi
    thr6 = np.stack([-t6hi, -t6lo]).astype(np.float16)   # [2, 4096]
    assert np.array_equal(thr6.astype(np.float64).sum(0), -t6u)

    w7r = np.zeros((16, 128, 2, 4096), np.float32)
    for kc in range(16):
        for j in range(2):
            w7r[kc, :, j, :] = w7[:, 256 * kc + 128 * j: 256 * kc + 128 * (j + 1)].T
    P['w7c'] = np.ascontiguousarray(
        w7r.reshape(16, 128, 2, 8, 512).transpose(3, 1, 0, 2, 4)
    ).reshape(8, 128, 16 * 1024).astype(FP8)

    s7, b7, m7 = bn_scale(inputs['bn7'])
    K7 = w7.sum(1)
    sc7 = (2.0 * s7).astype(np.float32).reshape(32, 128).T               # [128,32]
    bi7 = (((-K7 - m7) * s7) + b7).astype(np.float32).reshape(32, 128).T

    w8 = _f32(inputs['lin3_w'])
    w8t = np.zeros((32, 128, 1000), np.float32)
    for kc in range(32):
        w8t[kc] = w8[:, 128 * kc:128 * (kc + 1)].T
    P['w8c'] = np.ascontiguousarray(w8t.transpose(1, 0, 2)).reshape(
        128, 32 * 1000).astype(np.float16)
    b8 = _f32(inputs['lin3_b'])
    b8hi = b8.astype(np.float16).astype(np.float32)
    b8hl = np.stack([b8hi, b8 - b8hi]).astype(np.float16)                # [2,1000]

    gD = np.zeros((128, GD_LEN), np.float32)
    gD[0:96, GD_OFF['t1x']] = t1x
    gD[:, GD_OFF['thr2u']:GD_OFF['thr2u'] + 2] = thr2u
    gD[:, GD_OFF['thr3u']:GD_OFF['thr3u'] + 3] = thr3u
    gD[:, GD_OFF['thr4u']:GD_OFF['thr4u'] + 3] = thr4u
    gD[:, GD_OFF['thr5u']:GD_OFF['thr5u'] + 2] = thr5u
    gD[:, GD_OFF['sc7']:GD_OFF['sc7'] + 32] = sc7
    gD[:, GD_OFF['bi7']:GD_OFF['bi7'] + 32] = bi7
    gD[0:16, GD_OFF['id16f']:GD_OFF['id16f'] + 16] = np.eye(16, dtype=np.float32)
    P['gD'] = gD

    gE = np.zeros((2, GE_LEN), np.float16)
    gE[:, GE_OFF['thr6']:GE_OFF['thr6'] + 4096] = thr6
    gE[:, GE_OFF['b8']:GE_OFF['b8'] + 1000] = b8hl
    P['gE'] = gE

    P['id16'] = np.eye(16).astype(BF16)
    return P


IN_SPECS = [
    ('col_hi', (121, 3 * 3025), mybir.dt.float16, True),
    ('col_lo', (121, 3 * 3025), mybir.dt.float16, True),
    ('gA', (128, GA_LEN), mybir.dt.float8e4, False),
    ('gB', (96, GB_LEN), mybir.dt.float8e4, False),
    ('gC', (121, GC_LEN), mybir.dt.float16, False),
    ('gD', (128, GD_LEN), mybir.dt.float32, False),
    ('gE', (2, GE_LEN), mybir.dt.float16, False),
    ('w6c', (8, 128, 36 * 1024), mybir.dt.float8e4, False),
    ('w7c', (8, 128, 16 * 1024), mybir.dt.float8e4, False),
    ('w8c', (128, 32 * 1000), mybir.dt.float16, False),
    ('id16', (16, 16), mybir.dt.bfloat16, False),
]


def build_module(B, dbg=False):
    nc = bass.Bass("TRN2", target_bir_lowering=False, debug=False,
                   num_devices=NCORES)
    ins = {}
    for name, shp, dt, per_img in IN_SPECS:
        shape = (B,) + shp if per_img else shp
        ins[name] = nc.dram_tensor(name, list(shape), dt, kind="ExternalInput").ap()

    out = nc.dram_tensor("out", [B, 1000], mybir.dt.float32, kind="ExternalOutput").ap()
    dbg_outs = {}
    if dbg:
        for name, shape, dt in [
            ('d_u1', [B, 96, S2], mybir.dt.float8e4),
            ('d_u2', [B, 128, 2, S3], mybir.dt.float8e4),
            ('d_u3a', [B, 128, 2, S4], mybir.dt.float8e4),
            ('d_u3b', [B, 128, S4], mybir.dt.float8e4),
            ('d_u4a', [B, 128, 2, S4], mybir.dt.float8e4),
            ('d_u4b', [B, 128, S4], mybir.dt.float8e4),
            ('d_t5', [128, 2, 36, B], mybir.dt.float8e4),
            ('d_t6', [128, 16, 2, B], mybir.dt.float8e4),
        ]:
            dbg_outs[name] = nc.dram_tensor(name, shape, dt, kind="ExternalOutput").ap()

    dma_handles = []
    tail_extra = []

    def ldma(dst, src):        # im2col streaming queue (gpsimd)
        h = nc.gpsimd.dma_start(dst, src)
        dma_handles.append(h)
        return h

    def wdma(dst, src):        # weight streaming queue (sync/SP)
        h = nc.sync.dma_start(dst, src)
        dma_handles.append(h)
        return h

    def adma(dst, src):        # aux weight queue (scalar/ACT)
        h = nc.scalar.dma_start(dst, src)
        dma_handles.append(h)
        return h

    def odma(dst, src):
        h = nc.sync.dma_start(dst, src)
        dma_handles.append(h)
        return h

    with tile.TileContext(nc) as tc:
        with tc.tile_pool(name="wpool", bufs=1) as wp, \
             tc.tile_pool(name="cpool", bufs=1) as cp:

            # ---------- resident constant views ----------
            W = {}

            tD = wp.tile([128, GD_LEN], mybir.dt.float32, tag="gD")
            wdma(tD[:], ins['gD'][:])
            # fc-only constants (thr6/b8/id16) load on the aux queue so the
            # startup critical path is just gD -> conv weights -> first cols
            tE = wp.tile([2, GE_LEN], mybir.dt.float16, tag="gE")
            adma(tE[:], ins['gE'][:])
            tI = wp.tile([16, 16], mybir.dt.bfloat16, tag="id16")
            adma(tI[:], ins['id16'][:])
            W['id16'] = tI

            W['t1x'] = apv(tD[:], GD_OFF['t1x'], [[GD_LEN, 96], [1, 1]])
            for key, cnt in (('thr2u', 2), ('thr3u', 3), ('thr4u', 3),
                             ('thr5u', 2), ('sc7', 32), ('bi7', 32)):
                W[key] = [apv(tD[:], GD_OFF[key] + c, [[GD_LEN, 128], [1, 1]])
                          for c in range(cnt)]
            W['id16f'] = apv(tD[:], GD_OFF['id16f'], [[GD_LEN, 16], [1, 16]])
            W['thr6'] = [apv(tE[:], GE_OFF['thr6'] + n0, [[GE_LEN, 2], [1, 512]])
                         for n0 in range(0, 4096, 512)]
            W['b8'] = [apv(tE[:], GE_OFF['b8'] + h * 500, [[GE_LEN, 2], [1, 500]])
                       for h in range(2)]

            t_ones = wp.tile([2, 128], mybir.dt.float16, tag="ones")
            nc.vector.memset(t_ones[:], 1.0)

            t_zw = wp.tile([1, 128], mybir.dt.float8e4, tag="zw")
            nc.vector.memset(t_zw[:], 0.0)
            t_zr = wp.tile([1, 1024], mybir.dt.float8e4, tag="zr")
            nc.vector.memset(t_zr[:], 0.0)

            last_out = {}   # proc -> 1-cell AP of that engine's latest evac output

            # engine/queue intro: first-touch the packed-constant DMA queues
            scr = wp.tile([128, 1], mybir.dt.float32, tag="scr")
            nc.vector.tensor_copy(scr[:, 0:1], tD[:, 0:1])
            scrA = wp.tile([128, 1], mybir.dt.float32, tag="scrA")
            nc.scalar.copy(scrA[:, 0:1], tD[:, 0:1])
            scrE = wp.tile([2, 4], mybir.dt.float16, tag="scrE")
            nc.vector.tensor_copy(scrE[:, 0:1], tE[:, 0:1])
            scrEA = wp.tile([2, 4], mybir.dt.float16, tag="scrEA")
            nc.scalar.copy(scrEA[:, 0:1], tE[:, 0:1])

            # fc1 lhsT source, accumulated across all images
            Bp = max(16, B)   # fp8 DR slot strides must be 16B-aligned
            t5 = cp.tile([128, 2 * 36 * Bp], mybir.dt.float8e4, tag="t5")
            nc.vector.memset(t5[:], 0.0)

            # ================= conv phase =================
            with tc.tile_pool(name="cw", bufs=1) as cwp, \
                 tc.tile_pool(name="act", bufs=5) as ap_, \
                 tc.tile_pool(name="stage", bufs=1) as stp, \
                 tc.tile_pool(name="col", bufs=2) as colp, \
                 tc.tile_pool(name="ps1", bufs=2, space="PSUM") as ps1, \
                 tc.tile_pool(name="ps2", bufs=1, space="PSUM") as ps2, \
                 tc.tile_pool(name="ps3", bufs=4, space="PSUM") as ps3:

                # conv weights: three packed DMAs on the weight queue
                tC = cwp.tile([121, GC_LEN], mybir.dt.float16, tag="gC")
                wdma(tC[:], ins['gC'][:])
                tB = cwp.tile([96, GB_LEN], mybir.dt.float8e4, tag="gB")
                wdma(tB[:], ins['gB'][:])
                tA = cwp.tile([128, GA_LEN], mybir.dt.float8e4, tag="gA")
                wdma(tA[:], ins['gA'][:])

                for k, n in GC_ENTRIES:
                    W[k] = apv(tC[:], GC_OFF[k], [[GC_LEN, 121], [1, 96]])
                for k, n in GB_ENTRIES:
                    if n == 256:
                        W[k] = apv(tB[:], GB_OFF[k], [[GB_LEN, 96], [128, 2], [1, 128]])
                    else:
                        W[k] = apv(tB[:], GB_OFF[k], [[GB_LEN, 96], [1, 128]])
                for k, n in GA_ENTRIES:
                    if n == 256:
                        W[k] = apv(tA[:], GA_OFF[k], [[GA_LEN, 128], [128, 2], [1, 128]])
                    else:
                        W[k] = apv(tA[:], GA_OFF[k], [[GA_LEN, 128], [1, 128]])

                NB = 4
                for i0 in range(0, B, NB):
                    blk = list(range(i0, min(B, i0 + NB)))
                    u2s, u3as, u3bs, u4as, u4bs = {}, {}, {}, {}, {}
                    for i in blk:
                        # ---- conv1 ----
                        colh = colp.tile([121, 3 * 3025], mybir.dt.float16, tag="colh")
                        ldma(colh[:], ins['col_hi'][i])
                        coll = colp.tile([121, 3 * 3025], mybir.dt.float16, tag="coll")
                        ldma(coll[:], ins['col_lo'][i])

                        c1f = stp.tile([96, 3025], mybir.dt.float32, tag="c1f")
                        passes = [('w1_hi', colh), ('w1_hi', coll), ('w1_lo', colh)]
                        for n0 in range(0, 3025, 512):
                            n1 = min(3025, n0 + 512)
                            pc = ps1.tile([96, 512], mybir.dt.float32, tag="pc1")
                            cnt = 0
                            for ct in range(3):
                                for wk, colt in passes:
                                    cnt += 1
                                    nc.tensor.matmul(
                                        pc[:, 0:n1 - n0], W[wk + str(ct)],
                                        apv(colt[:], ct * 3025 + n0,
                                            [[3 * 3025, 121], [1, n1 - n0]]),
                                        start=(cnt == 1), stop=(cnt == 9))
                            last_out['ACT'] = nc.scalar.copy(c1f[:, n0:n1], pc[:, 0:n1 - n0])

                        # pool1: 55x55 -> 27x27
                        pm1 = stp.tile([96, 55 * 27], mybir.dt.float32, tag="pm1")
                        d = [[3025, 96], [55, 55], [2, 27]]
                        o = [[55 * 27, 96], [27, 55], [1, 27]]
                        nc.vector.tensor_max(apv(pm1[:], 0, o), apv(c1f[:], 0, d), apv(c1f[:], 1, d))
                        nc.vector.tensor_max(apv(pm1[:], 0, o), apv(pm1[:], 0, o), apv(c1f[:], 2, d))
                        po1 = stp.tile([96, 729], mybir.dt.float32, tag="po1")
                        d2 = [[55 * 27, 96], [54, 27], [1, 27]]
                        o2 = [[729, 96], [27, 27], [1, 27]]
                        nc.vector.tensor_max(apv(po1[:], 0, o2), apv(pm1[:], 0, d2), apv(pm1[:], 27, d2))
                        nc.vector.tensor_max(apv(po1[:], 0, o2), apv(po1[:], 0, o2), apv(pm1[:], 54, d2))

                        u1 = ap_.tile([96, S2], mybir.dt.float8e4, tag="u1")
                        nc.vector.memset(u1[:], 0.0)
                        last_out['DVE'] = nc.vector.tensor_scalar(
                            apv(u1[:], B2, [[S2, 96], [G2, 27], [1, 27]]),
                            apv(po1[:], 0, o2), W['t1x'], None,
                            mybir.AluOpType.is_ge)
                        if dbg:
                            odma(dbg_outs['d_u1'][i], u1[:])

                        # ---- conv2 ----
                        u2 = ap_.tile([128, 2 * S3], mybir.dt.float8e4, tag="u2")
                        nc.vector.memset(u2[:], 0.0)
                        for mt in range(2):
                            p2 = ps2.tile([128, N2], mybir.dt.float32, tag="p2")
                            nmm = 0
                            tot = 5 * 3 * 2
                            for kx in range(5):
                                dx = kx - 2
                                for kg in range(2):
                                    off = B2 + (2 * kg - 2) * G2 + dx
                                    for n0, n1 in ((0, 512), (512, 864)):
                                        nmm += 1
                                        nc.tensor.matmul(
                                            p2[:, n0:n1], W[f'w2dr{mt}{kx}{kg}'],
                                            apv(u1[:], off + n0,
                                                [[S2, 96], [G2, 2], [1, n1 - n0]]),
                                            start=(kx == 0 and kg == 0),
                                            stop=(nmm >= tot - 1), perf_mode=DR)
                                off = B2 + 2 * G2 + dx
                                for n0, n1 in ((0, 512), (512, 864)):
                                    nmm += 1
                                    # ky=4 paired with an all-zero phantom row
                                    nc.tensor.matmul(
                                        p2[:, n0:n1], W[f'w2sr{mt}{kx}'],
                                        apv(u1[:], off + n0, [[S2, 96], [G2, 2], [1, n1 - n0]]),
                                        start=False, stop=(nmm >= tot - 1),
                                        perf_mode=DR)
                            c2f = stp.tile([128, N2], mybir.dt.float32, tag="c2f")
                            last_out['ACT'] = nc.scalar.copy(c2f[:], p2[:])
                            pm2 = stp.tile([128, 27 * 13], mybir.dt.float32, tag="pm2")
                            d = [[N2, 128], [G2, 27], [2, 13]]
                            o = [[27 * 13, 128], [13, 27], [1, 13]]
                            nc.vector.tensor_max(apv(pm2[:], 0, o), apv(c2f[:], 0, d), apv(c2f[:], 1, d))
                            nc.vector.tensor_max(apv(pm2[:], 0, o), apv(pm2[:], 0, o), apv(c2f[:], 2, d))
                            po2 = stp.tile([128, 169], mybir.dt.float32, tag="po2")
                            d2 = [[27 * 13, 128], [26, 13], [1, 13]]
                            o2 = [[169, 128], [13, 13], [1, 13]]
                            nc.vector.tensor_max(apv(po2[:], 0, o2), apv(pm2[:], 0, d2), apv(pm2[:], 13, d2))
                            nc.vector.tensor_max(apv(po2[:], 0, o2), apv(po2[:], 0, o2), apv(pm2[:], 26, d2))
                            last_out['DVE'] = nc.vector.tensor_scalar(
                                apv(u2[:], mt * S3 + B3, [[2 * S3, 128], [G3, 13], [1, 13]]),
                                apv(po2[:], 0, o2), W['thr2u'][mt], None,
                                mybir.AluOpType.is_ge)
                        u2s[i] = u2
                        if dbg:
                            odma(dbg_outs['d_u2'][i], u2[:].rearrange("p (a b) -> p a b", a=2))

                    # ---- conv3: image-blocked (shared weight loads) ----
                    for i in blk:
                        u3a = ap_.tile([128, 2 * S4], mybir.dt.float8e4, tag="u3a")
                        nc.vector.memset(u3a[:], 0.0)
                        u3b = ap_.tile([128, S4], mybir.dt.float8e4, tag="u3b")
                        nc.vector.memset(u3b[:], 0.0)
                        u3as[i], u3bs[i] = u3a, u3b
                    for mt in range(3):
                        p3s = {}
                        for i in blk:
                            p3f = ps3.tile([128, 512], mybir.dt.float32, tag="p45")
                            p3s[i] = p3f[:, 0:N3]
                        for ky in range(3):
                            for kx in range(3):
                                off = B3 + (ky - 1) * G3 + (kx - 1)
                                for i in blk:
                                    nc.tensor.matmul(
                                        p3s[i], W[f'w3dr{mt}{ky}{kx}'],
                                        apv(u2s[i][:], off, [[2 * S3, 128], [S3, 2], [1, N3]]),
                                        start=(ky == 0 and kx == 0),
                                        stop=(ky == 2 and kx == 2),
                                        perf_mode=DR)
                        for i in blk:
                            srcp = apv(p3s[i], 0, [[512, 128], [G3, 13], [1, 13]])
                            if mt < 2:
                                dstu = apv(u3as[i][:], mt * S4 + B4, [[2 * S4, 128], [G4, 13], [1, 13]])
                            else:
                                dstu = apv(u3bs[i][:], B4, [[S4, 128], [G4, 13], [1, 13]])
                            last_out['DVE'] = nc.vector.tensor_scalar(
                                dstu, srcp, W['thr3u'][mt],
                                None, mybir.AluOpType.is_ge)
                    if dbg:
                        for i in blk:
                            odma(dbg_outs['d_u3a'][i], u3as[i][:].rearrange("p (a b) -> p a b", a=2))
                            odma(dbg_outs['d_u3b'][i], u3bs[i][:])

                    # ---- conv4 / conv5: image-blocked ----
                    for i in blk:
                        u4a = ap_.tile([128, 2 * S4], mybir.dt.float8e4, tag="u4a")
                        nc.vector.memset(u4a[:], 0.0)
                        u4b = ap_.tile([128, S4], mybir.dt.float8e4, tag="u4b")
                        nc.vector.memset(u4b[:], 0.0)
                        u4as[i], u4bs[i] = u4a, u4b

                    def conv45blk(nm, nmt, uas, ubs, sink):
                        for mt in range(nmt):
                            p4s = {}
                            for i in blk:
                                p4f = ps3.tile([128, 512], mybir.dt.float32, tag="p45")
                                p4s[i] = p4f[:, 0:N4]
                            for ky in range(3):
                                for kx in range(3):
                                    off = B4 + (ky - 1) * G4 + (kx - 1)
                                    for i in blk:
                                        nc.tensor.matmul(
                                            p4s[i], W[f'{nm}a{mt}{ky}{kx}'],
                                            apv(uas[i][:], off, [[2 * S4, 128], [S4, 2], [1, N4]]),
                                            start=(ky == 0 and kx == 0),
                                            stop=False, perf_mode=DR)
                            for kx in range(3):
                                dx = kx - 1
                                for i in blk:
                                    nc.tensor.matmul(
                                        p4s[i], W[f'{nm}bd{mt}{kx}'],
                                        apv(ubs[i][:], B4 - G4 + dx, [[S4, 128], [G4, 2], [1, N4]]),
                                        start=False, stop=False, perf_mode=DR)
                            for kx in range(3):
                                dx = kx - 1
                                for i in blk:
                                    # ky=2 paired with an all-zero phantom row
                                    nc.tensor.matmul(
                                        p4s[i], W[f'{nm}bs{mt}{kx}'],
                                        apv(ubs[i][:], B4 + G4 + dx, [[S4, 128], [G4, 2], [1, N4]]),
                                        start=False, stop=(kx == 2), perf_mode=DR)
                            for i in blk:
                                sink(mt, i, p4s[i])

                    def sink4(mt, i, p4):
                        srcp = apv(p4, 0, [[512, 128], [G4, 13], [1, 13]])
                        if mt < 2:
                            dstu = apv(u4as[i][:], mt * S4 + B4, [[2 * S4, 128], [G4, 13], [1, 13]])
                        else:
                            dstu = apv(u4bs[i][:], B4, [[S4, 128], [G4, 13], [1, 13]])
                        last_out['DVE'] = nc.vector.tensor_scalar(
                            dstu, srcp, W['thr4u'][mt],
                            None, mybir.AluOpType.is_ge)

                    conv45blk('w4', 3, u3as, u3bs, sink4)
                    if dbg:
                        for i in blk:
                            odma(dbg_outs['d_u4a'][i], u4as[i][:].rearrange("p (a b) -> p a b", a=2))
                            odma(dbg_outs['d_u4b'][i], u4bs[i][:])

                    def sink5(mt, i, p5):
                        c5f = stp.tile([128, N4], mybir.dt.float32, tag="c5f")
                        last_out['ACT'] = nc.scalar.copy(c5f[:], p5)
                        pm5 = stp.tile([128, 13 * 6], mybir.dt.float32, tag="pm5")
                        d = [[N4, 128], [G4, 13], [2, 6]]
                        o = [[13 * 6, 128], [6, 13], [1, 6]]
                        nc.vector.tensor_max(apv(pm5[:], 0, o), apv(c5f[:], 0, d), apv(c5f[:], 1, d))
                        nc.vector.tensor_max(apv(pm5[:], 0, o), apv(pm5[:], 0, o), apv(c5f[:], 2, d))
                        po5 = stp.tile([128, 36], mybir.dt.float32, tag="po5")
                        d2 = [[13 * 6, 128], [12, 6], [1, 6]]
                        o2 = [[36, 128], [6, 6], [1, 6]]
                        nc.vector.tensor_max(apv(po5[:], 0, o2), apv(pm5[:], 0, d2), apv(pm5[:], 6, d2))
                        nc.vector.tensor_max(apv(po5[:], 0, o2), apv(po5[:], 0, o2), apv(pm5[:], 12, d2))
                        h5 = nc.vector.tensor_scalar(
                            apv(t5[:], mt * 36 * Bp + i, [[2 * 36 * Bp, 128], [Bp, 36]]),
                            po5[:], W['thr5u'][mt], None,
                            mybir.AluOpType.is_ge)
                        last_out['DVE'] = h5
                        if i == B - 1:
                            tail_extra.append(h5)

                    conv45blk('w5', 2, u4as, u4bs, sink5)

            if dbg:
                odma(dbg_outs['d_t5'][:],
                     t5[:].rearrange("p (a b c) -> p a b c", a=2, b=36)
                     if Bp == B else
                     apv(t5[:], 0, [[2 * 36 * Bp, 128], [36 * Bp, 2], [Bp, 36], [1, B]]))

            # ================= fc phase =================
            with tc.tile_pool(name="fcw", bufs=2) as fcw, \
                 tc.tile_pool(name="fc8", bufs=1) as fc8p, \
                 tc.tile_pool(name="psf", bufs=2, space="PSUM") as psf, \
                 tc.tile_pool(name="pst", bufs=2, space="PSUM") as pst, \
                 tc.tile_pool(name="ps3f", bufs=1, space="PSUM") as ps3f:

                # fc3 weights stream on the aux queue while fc1 runs
                # (piece-split and dep-interleaved variants both simulated
                # slower; the scheduler does best with one transfer here)
                w8t_t = fc8p.tile([128, 32 * 1000], mybir.dt.float16, tag="w8c")
                adma(w8t_t[:], ins['w8c'][:])

                # fc1 (by output chunks of 512); weights as one contiguous
                # chunk DMA each, double-buffered on the weight queue
                t6b = []
                for n in range(8):
                    t6b_n = cp.tile([B, 512], mybir.dt.bfloat16, tag=f"t6b{n}")
                    t6b.append(t6b_n)
                for nchi in range(8):
                    wc = fcw.tile([128, 36 * 1024], mybir.dt.float8e4, tag="w6c")
                    # split each chunk across the two idle DMA queues
                    wdma(apv(wc[:], 0, [[36 * 1024, 128], [1, 18 * 1024]]),
                         apv(ins['w6c'][nchi], 0, [[36 * 1024, 128], [1, 18 * 1024]]))
                    ldma(apv(wc[:], 18 * 1024, [[36 * 1024, 128], [1, 18 * 1024]]),
                         apv(ins['w6c'][nchi], 18 * 1024, [[36 * 1024, 128], [1, 18 * 1024]]))
                    pf = psf.tile([B, 512], mybir.dt.float32, tag="pf")
                    for s in range(36):
                        nc.tensor.matmul(
                            pf[:], apv(t5[:], s * Bp, [[2 * 36 * Bp, 128], [36 * Bp, 2], [1, B]]),
                            apv(wc[:], s * 1024, [[36 * 1024, 128], [512, 2], [1, 512]]),
                            start=(s == 0), stop=False, perf_mode=DR)
                    nc.tensor.matmul(pf[:], t_ones[:, 0:B], W['thr6'][nchi],
                                     start=False, stop=True)
                    last_out['DVE'] = nc.vector.tensor_scalar(
                        t6b[nchi][:], pf[:], 0.0, None, mybir.AluOpType.is_ge)

                # transpose to t6[kc] tiles [128, (j, B)]
                t6 = []
                for kc in range(16):
                    t6_kc = cp.tile([128, 2 * Bp], mybir.dt.float8e4, tag=f"t6_{kc}")
                    t6.append(t6_kc)
                Ba = ((B + 1) // 2) * 2   # 4-byte-aligned slot stride for bf16
                for kc in range(16):
                    pt2f = pst.tile([128, 1024], mybir.dt.bfloat16, tag="ptr")
                    for j in range(2):
                        src_col = 256 * kc + 128 * j
                        nc.tensor.transpose(
                            pt2f[:, j * Ba:j * Ba + B],
                            t6b[src_col // 512][:, src_col % 512: src_col % 512 + 128],
                            W['id16'][0:B, 0:B])
                    last_out['ACT'] = nc.scalar.copy(
                        apv(t6[kc][:], 0, [[2 * Bp, 128], [Bp, 2], [1, B]]),
                        apv(pt2f[:], 0, [[1024, 128], [Ba, 2], [1, B]]))
                if dbg:
                    for kc in range(16):
                        odma(apv(dbg_outs['d_t6'][:], kc * 2 * B,
                                 [[16 * 2 * B, 128], [B, 2], [1, B]]),
                             apv(t6[kc][:], 0, [[2 * Bp, 128], [Bp, 2], [1, B]]))

                # fc2
                s7f = []
                for n in range(8):
                    s7f_n = cp.tile([B, 512], mybir.dt.float32, tag=f"s7f{n}")
                    s7f.append(s7f_n)
                for nchi in range(8):
                    # reuse the fc1 chunk ring (same tag/shape): fc2 chunks
                    # occupy the first 16K elements of a w6c-shaped tile
                    wc = fcw.tile([128, 36 * 1024], mybir.dt.float8e4, tag="w6c")
                    wdma(apv(wc[:], 0, [[36 * 1024, 128], [1, 8 * 1024]]),
                         apv(ins['w7c'][nchi], 0, [[16 * 1024, 128], [1, 8 * 1024]]))
                    ldma(apv(wc[:], 8 * 1024, [[36 * 1024, 128], [1, 8 * 1024]]),
                         apv(ins['w7c'][nchi], 8 * 1024, [[16 * 1024, 128], [1, 8 * 1024]]))
                    pf = psf.tile([B, 512], mybir.dt.float32, tag="pf")
                    for kc in range(16):
                        nc.tensor.matmul(
                            pf[:], apv(t6[kc][:], 0, [[2 * Bp, 128], [Bp, 2], [1, B]]),
                            apv(wc[:], kc * 1024, [[36 * 1024, 128], [512, 2], [1, 512]]),
                            start=(kc == 0), stop=(kc == 15), perf_mode=DR)
                    last_out['DVE'] = nc.vector.tensor_copy(s7f[nchi][:], pf[:])

                # transpose + bn7-relu -> y7 [128, (kc, B)] fp16
                y7 = []
                for kc in range(32):
                    y7_kc = cp.tile([128, B], mybir.dt.float16, tag=f"y7_{kc}")
                    y7.append(y7_kc)
                for kc in range(32):
                    ptf = pst.tile([128, 512], mybir.dt.float32, tag="ptrf")
                    pt = ptf[:, 0:B]
                    col = 128 * kc
                    nc.tensor.transpose(pt, s7f[col // 512][:, col % 512: col % 512 + 128],
                                        W['id16f'][0:B, 0:B])
                    hact = nc.scalar.activation(
                        y7[kc][:], pt,
                        mybir.ActivationFunctionType.Relu,
                        bias=W['bi7'][kc], scale=W['sc7'][kc])
                    last_out['ACT'] = hact

                # fc3
                pf3af = ps3f.tile([B, 512], mybir.dt.float32, tag="pf3a")
                pf3a = pf3af[:, 0:500]
                pf3bf = ps3f.tile([B, 512], mybir.dt.float32, tag="pf3b")
                pf3b = pf3bf[:, 0:500]
                for kc in range(32):
                    lhs = y7[kc][:]
                    nc.tensor.matmul(pf3a, lhs,
                                     apv(w8t_t[:], kc * 1000, [[32 * 1000, 128], [1, 500]]),
                                     start=(kc == 0), stop=False)
                    nc.tensor.matmul(pf3b, lhs,
                                     apv(w8t_t[:], kc * 1000 + 500, [[32 * 1000, 128], [1, 500]]),
                                     start=(kc == 0), stop=False)
                nc.tensor.matmul(pf3a, t_ones[:, 0:B], W['b8'][0],
                                 start=False, stop=True)
                nc.tensor.matmul(pf3b, t_ones[:, 0:B], W['b8'][1],
                                 start=False, stop=True)

                of = cp.tile([B, 1000], mybir.dt.float32, tag="of")
                nc.vector.tensor_copy(of[:, 0:500], pf3a)
                h_of = nc.vector.tensor_copy(of[:, 500:1000], pf3b)
                tail_extra.append(h_of)
                dma_handles.append(nc.sync.dma_start(out[:], of[:]))

            # ---------- tail-sync for the final drain ----------
            for h in dma_handles[-40:] + tail_extra:
                n = nc.sync.nop(nofuse=True)
                add_dep_helper(n.ins, h.ins, reason="tail drain sync")

    legalize_waits(nc)
    return nc


def legalize_waits(nc):
    """Split multi-wait sync lists into single-wait same-engine NOPs.

    TPB instructions (compute, NOP, drain, DMA pseudo-ops) accept one
    sync-wait command in this walrus; extra waits are moved onto freshly
    inserted NOPs placed directly before the instruction in its basic block
    (same engine stream).
    """
    f = nc.m.functions[0]
    ctr = 0
    ndedup = 0
    for blk in f.blocks:
        new = []
        last_lw_key = None
        for inst in blk.instructions:
            tname0 = type(inst).__name__
            if tname0 == 'InstLdweights':
                si0 = inst.sync_info
                key = (str(inst.ins[0]), str(inst.perf_mode))
                if False and key == last_lw_key and not (si0 and si0.on_wait):
                    ndedup += 1
                    continue          # redundant reload of identical weights
                last_lw_key = key
            si = inst.sync_info
            tname = type(inst).__name__
            if si is not None and inst.engine is not None:
                waits = list(si.on_wait)
                if len(waits) > 1:
                    for w in waits[:-1]:
                        ctr += 1
                        n = mybir.InstNoOp(name=f"I-wfix{ctr}", ins=[], outs=[])
                        n.engine = inst.engine
                        n.sync_info = bass_rust.SyncInfo(on_wait=[w], on_update=[])
                        new.append(n)
                    inst.sync_info = bass_rust.SyncInfo(
                        on_wait=[waits[-1]], on_update=list(si.on_update))
            new.append(inst)
        blk.instructions = new
    return ctr, ndedup


# ======================= entry point =======================

def make_in_maps(P, B):
    in_maps = []
    for c in range(NCORES):
        m = {}
        for name, shp, dt, per_img in IN_SPECS:
            a = P[name]
            if per_img:
                a = a[c * B:(c + 1) * B]
            m[name] = np.ascontiguousarray(a)
        in_maps.append(m)
    return in_maps


def kernel(**inputs) -> np.ndarray:
    P = prep_host(inputs)
    B = P['col_hi'].shape[0] // NCORES
    nc = build_module(B, dbg=False)
    in_maps = make_in_maps(P, B)
    res = run_bass_kernel_spmd(nc, in_maps, core_ids=list(range(NCORES)))
    outs = [res.results[c]['out'] for c in range(NCORES)]
    return np.concatenate(outs, axis=0).astype(np.float32)
